# Initial kernel scaffold
#
"""MHA kernel for TRN2, 8 NeuronCores.

Sharding: core c = b*4 + g handles batch b (of 2) and head-group g (4 of 16
heads, contiguous head-dim columns 512g:512g+512).  Each core computes
  QT/KT = (W[cols,:] @ x_b.T) with RoPE applied   -> [512, 2048] head-dim major
  V     = x_b @ Wv[cols,:].T                      -> [2048, 512]
  causal attention per head in transposed-score layout (no-max softmax;
  scores ~ N(0,1) so exp never overflows)
  partial_out = O_part @ Wo[:, cols].T            -> [2048, 2048]
Host sums the 4 partials per batch.

Matmuls run in bf16 (1 cyc/row on PE); accumulation is fp32 in PSUM.
Elementwise work stays on ACT/DVE only (Pool TT hits the ISA sync-wait
slot limit when an op depends on 3+ engines).
"""

import math

import numpy as np
import ml_dtypes

import concourse.bass as bass
import concourse.mybir as mybir
import concourse.tile as tile
from concourse.bass_utils import run_bass_kernel_spmd

S = 2048
D = 2048
HD = 128  # head dim
NHC = 4  # heads per core
DH = NHC * HD  # 512 head-dim columns per core
NKT = D // 128  # 16 contraction k-tiles
SB = 512  # S block for free dims
NQB = S // SB  # 4 q blocks
F32 = mybir.dt.float32
BF16 = mybir.dt.bfloat16
NPBF16 = ml_dtypes.bfloat16

_CACHE = {}


def build_bass():
    nc = bass.Bass()
    xT = nc.declare_dram_parameter("xT", [D, S], BF16, isOutput=False)
    wqT = nc.declare_dram_parameter("wqT", [D, DH], BF16, isOutput=False)
    wkT = nc.declare_dram_parameter("wkT", [D, DH], BF16, isOutput=False)
    wvT = nc.declare_dram_parameter("wvT", [D, DH], BF16, isOutput=False)
    woT = nc.declare_dram_parameter("woT", [DH, D], BF16, isOutput=False)
    cosf = nc.declare_dram_parameter("cosf", [HD, S], BF16, isOutput=False)
    sinsg = nc.declare_dram_parameter("sinsg", [HD, S], BF16, isOutput=False)
    pswap_d = nc.declare_dram_parameter("pswap", [HD, HD], BF16, isOutput=False)
    binmask_d = nc.declare_dram_parameter(
        "binmask", [4 * 128, SB], BF16, isOutput=False
    )
    out_d = nc.declare_dram_parameter("out", [S, D], F32, isOutput=True)

    with tile.TileContext(nc) as tc:
        with (
            tc.tile_pool(name="psum", bufs=1, space="PSUM") as psum,
            tc.tile_pool(name="main", bufs=1) as mp,
        ):
            # tiny constants first (zero-wait DVE ops at program start)
            ones_col = mp.tile([128, 1], F32, name="ones_col")
            nc.vector.memset(ones_col[:, :], 1.0)
            ones_row = mp.tile([1, 128], F32, name="ones_row")
            nc.vector.memset(ones_row[:, :], 1.0)
            dscr = mp.tile([1, 1], F32, name="dscr")
            _tmpl_dve = nc.vector.memset(dscr[:, :], 0.0)
            _tmpl_act = nc.scalar.copy(dscr[:, :], dscr[:, :])
            _CACHE["tmpl"] = {"DVE": _tmpl_dve.ins, "Activation": _tmpl_act.ins}

            # persistent bf16 tensors: QT/KT per head, V per s-tile, OT per head
            qts = [mp.tile([128, S], BF16, name=f"qt{h}", tag="qt", bufs=NHC)
                   for h in range(NHC)]
            kts = [mp.tile([128, S], BF16, name=f"kt{h}", tag="kt", bufs=NHC)
                   for h in range(NHC)]
            vts = [mp.tile([128, DH], BF16, name=f"v{st}", tag="v", bufs=NKT)
                   for st in range(NKT)]
            ots = [mp.tile([128, S], BF16, name=f"ot{h}", tag="ot", bufs=NHC)
                   for h in range(NHC)]

            # ---------------- phase 1: projections + RoPE ------------------
            with tc.tile_pool(name="ph1", bufs=1) as p1:
                cos_t = p1.tile([HD, S], BF16, name="cos_t")
                sin_t = p1.tile([HD, S], BF16, name="sin_t")
                psw_t = p1.tile([HD, HD], BF16, name="psw_t")
                nc.sync.dma_start(out=cos_t[:, :], in_=cosf[:, :])
                nc.sync.dma_start(out=sin_t[:, :], in_=sinsg[:, :])
                nc.sync.dma_start(out=psw_t[:, :], in_=pswap_d[:, :])
                # DVE touches so later DVE consumers carry own-engine deps
                nc.vector.tensor_copy(cos_t[:, :], cos_t[:, :])
                nc.vector.tensor_copy(sin_t[:, :], sin_t[:, :])

                # xT fully resident: 16 bf16 tiles [128, 2048]
                xts = []
                for kt in range(NKT):
                    xt = p1.tile([128, S], BF16, name=f"xt{kt}", tag="xt", bufs=NKT)
                    nc.sync.dma_start(
                        out=xt[:, :], in_=xT[kt * 128 : (kt + 1) * 128, :]
                    )
                    xts.append(xt)

                # --- V first ---
                wvts = []
                for kt in range(NKT):
                    wv = p1.tile([128, DH], BF16, name=f"wv{kt}", tag="wv", bufs=NKT)
                    nc.sync.dma_start(
                        out=wv[:, :], in_=wvT[kt * 128 : (kt + 1) * 128, :]
                    )
                    wvts.append(wv)
                for st in range(NKT):
                    ps = psum.tile([128, DH], F32, name=f"pv{st}", tag="pA", bufs=3)
                    for kt in range(NKT):
                        nc.tensor.matmul(
                            ps[:, :],
                            xts[kt][:, st * 128 : (st + 1) * 128],
                            wvts[kt][:, :],
                            start=(kt == 0),
                            stop=(kt == NKT - 1),
                        )
                    nc.scalar.copy(vts[st][:, :], ps[:, :])

                # --- Q and K per head: out[hd, S] with RoPE ---
                for h in range(NHC):
                    for proj, wsrc, dsts in (("k", wkT, kts), ("q", wqT, qts)):
                        wt = p1.tile(
                            [128, NKT * 128], BF16, name=f"w_{proj}{h}",
                            tag="wt", bufs=2,
                        )
                        for kt in range(NKT):
                            nc.sync.dma_start(
                                out=wt[:, kt * 128 : (kt + 1) * 128],
                                in_=wsrc[
                                    kt * 128 : (kt + 1) * 128,
                                    h * 128 : (h + 1) * 128,
                                ],
                            )
                        stage = p1.tile(
                            [128, S], BF16, name=f"st_{proj}{h}", tag="stage", bufs=2
                        )
                        for sb in range(NQB):
                            sl = slice(sb * SB, (sb + 1) * SB)
                            ps = psum.tile(
                                [128, SB], F32, name=f"pp{proj}{h}{sb}",
                                tag="pA", bufs=3,
                            )
                            for kt in range(NKT):
                                nc.tensor.matmul(
                                    ps[:, :],
                                    wt[:, kt * 128 : (kt + 1) * 128],
                                    xts[kt][:, sl],
                                    start=(kt == 0),
                                    stop=(kt == NKT - 1),
                                )
                            nc.scalar.copy(stage[:, sl], ps[:, :])
                            # rot = stage*cos + (pswap@stage)*sinsg -> bf16
                            psw = psum.tile(
                                [128, SB], F32, name=f"psw{proj}{h}{sb}",
                                tag="pB", bufs=2,
                            )
                            nc.tensor.matmul(
                                psw[:, :], psw_t[:, :], stage[:, sl],
                                start=True, stop=True,
                            )
                            tmp = p1.tile(
                                [128, SB], F32, name=f"tmp{proj}{h}{sb}",
                                tag="ropetmp", bufs=2,
                            )
                            tsin = p1.tile(
                                [128, SB], F32, name=f"tsin{proj}{h}{sb}",
                                tag="ropetsin", bufs=2,
                            )
                            nc.vector.tensor_tensor(
                                tmp[:, :], stage[:, sl], cos_t[:, sl],
                                mybir.AluOpType.mult,
                            )
                            nc.vector.tensor_tensor(
                                tsin[:, :], psw[:, :], sin_t[:, sl],
                                mybir.AluOpType.mult,
                            )
                            nc.vector.tensor_tensor(
                                dsts[h][:, sl], tsin[:, :], tmp[:, :],
                                mybir.AluOpType.add,
                            )

            # all-engine sync so phase-2 tiles reusing phase-1 addresses
            # don't accumulate per-engine catch-up waits
            tc.strict_bb_all_engine_barrier()

            # ---------------- phase 2: attention per head -------------------
            with tc.tile_pool(name="ph2", bufs=1) as p2:
                masks = []
                for j in range(4):
                    mk = p2.tile([128, SB], BF16, name=f"mask{j}", tag="mask", bufs=4)
                    nc.sync.dma_start(
                        out=mk[:, :], in_=binmask_d[j * 128 : (j + 1) * 128, :]
                    )
                    # DVE touch: later DVE consumers see an own-engine dep
                    nc.vector.tensor_copy(mk[:, :], mk[:, :])
                    masks.append(mk)

                for h in range(NHC):
                    for qb in range(NQB):
                        qsl = slice(qb * SB, (qb + 1) * SB)
                        nkt = 4 * (qb + 1)
                        pot = psum.tile(
                            [128, SB], F32, name=f"pot{h}{qb}", tag="pB", bufs=2
                        )
                        dacc = p2.tile(
                            [128, SB], F32, name=f"dacc{h}{qb}", tag="dacc", bufs=2
                        )
                        for kt in range(nkt):
                            pst = psum.tile(
                                [128, SB], F32, name=f"pst{h}{qb}{kt}",
                                tag="pA", bufs=3,
                            )
                            nc.tensor.matmul(
                                pst[:, :],
                                kts[h][:, kt * 128 : (kt + 1) * 128],
                                qts[h][:, qsl],
                                start=True,
                                stop=True,
                                skip_group_check=True,
                            )
                            es = p2.tile(
                                [128, SB], BF16, name=f"es{h}{qb}{kt}",
                                tag="es", bufs=17,
                            )
                            nc.scalar.activation(
                                es[:, :], pst[:, :], mybir.ActivationFunctionType.Exp
                            )
                            if kt >= 4 * qb:  # diagonal tile -> causal mask
                                nc.vector.tensor_tensor(
                                    es[:, :], es[:, :], masks[kt - 4 * qb][:, :],
                                    mybir.AluOpType.mult,
                                )
                            if kt == 0:
                                nc.vector.tensor_copy(dacc[:, :], es[:, :])
                            else:
                                nc.vector.tensor_tensor(
                                    dacc[:, :], dacc[:, :], es[:, :],
                                    mybir.AluOpType.add,
                                )
                            nc.tensor.matmul(
                                pot[:, :],
                                vts[kt][:, h * 128 : (h + 1) * 128],
                                es[:, :],
                                start=(kt == 0),
                                stop=(kt == nkt - 1),
                                skip_group_check=True,
                            )
                        # denom = colsum(dacc) over partitions -> [1, SB]
                        pden = psum.tile(
                            [1, SB], F32, name=f"pden{h}{qb}", tag="pC", bufs=1
                        )
                        nc.tensor.matmul(
                            pden[:, :], ones_col[:, :], dacc[:, :],
                            start=True, stop=True, skip_group_check=True,
                        )
                        recip = p2.tile(
                            [1, SB], F32, name=f"rc{h}{qb}", tag="recip", bufs=2
                        )
                        nc.vector.reciprocal(recip[:, :], pden[:, :])
                        pbc = psum.tile(
                            [128, SB], F32, name=f"pbc{h}{qb}", tag="pD", bufs=1
                        )
                        nc.tensor.matmul(
                            pbc[:, :], ones_row[:, :], recip[:, :],
                            start=True, stop=True, skip_group_check=True,
                        )
                        nc.scalar.copy(ots[h][:, qsl], pot[:, :])
                        # dummy DVE read of pbc absorbs the PE wait so the
                        # normalize mult only waits on ACT (1-wait TT limit)
                        nc.vector.tensor_copy(dscr[:, :], pbc[0:1, 0:1])
                        nc.vector.tensor_tensor(
                            ots[h][:, qsl], ots[h][:, qsl], pbc[:, :],
                            mybir.AluOpType.mult,
                        )

                # ------------- phase 3: output projection -------------------
                with tc.tile_pool(name="ph3", bufs=1) as p3:
                    wos = []
                    for h in range(NHC):
                        wo = p3.tile([128, D], BF16, name=f"wo{h}", tag="wo", bufs=NHC)
                        nc.sync.dma_start(
                            out=wo[:, :], in_=woT[h * 128 : (h + 1) * 128, :]
                        )
                        wos.append(wo)
                    for st in range(NKT):
                        osb = p3.tile([128, D], F32, name=f"osb{st}", tag="osb", bufs=2)
                        for nb in range(NQB):
                            po = psum.tile(
                                [128, SB], F32, name=f"po{st}{nb}", tag="pA", bufs=3
                            )
                            for h in range(NHC):
                                nc.tensor.matmul(
                                    po[:, :],
                                    ots[h][:, st * 128 : (st + 1) * 128],
                                    wos[h][:, nb * SB : (nb + 1) * SB],
                                    start=(h == 0),
                                    stop=(h == NHC - 1),
                                )
                            nc.scalar.copy(osb[:, nb * SB : (nb + 1) * SB], po[:, :])
                        nc.sync.dma_start(
                            out=out_d[st * 128 : (st + 1) * 128, :], in_=osb[:, :]
                        )
    _legalize_waits(nc)
    return nc


def _legalize_waits(nc):
    """Walrus TT/ACT structs hold only ONE sync wait.  Split excess waits
    onto cloned 1-element carrier ops inserted just before, same queue."""
    import copy

    tmpl = _CACHE["tmpl"]
    n = [0]

    def carrier(eng_name, wait, eng=None):
        n[0] += 1
        if eng_name == "PE":
            c = mybir.InstNoOp(name=f"I-legal-{n[0]}")
            c.engine = eng
        else:
            c = copy.deepcopy(tmpl[eng_name])
            c.name = f"I-legal-{n[0]}"
        c.sync_info = mybir.SyncInfo(on_wait=[wait], on_update=[])
        return c

    for f in nc.m.functions:
        for blk in f.blocks:
            new = []
            for inst in blk.instructions:
                si = getattr(inst, "sync_info", None)
                eng = str(getattr(inst, "engine", ""))
                tname = type(inst).__name__
                if (
                    si is not None
                    and len(si.on_wait) > 1
                    and tname not in ("InstEventSemaphore",)
                ):
                    if "DVE" in eng or "Pool" in eng:
                        key = "DVE"
                    elif "Activation" in eng:
                        key = "Activation"
                    else:
                        key = "PE"
                    waits = list(si.on_wait)
                    for w in waits[:-1]:
                        new.append(carrier(key, w, getattr(inst, "engine", None)))
                    inst.sync_info = mybir.SyncInfo(
                        on_wait=[waits[-1]], on_update=list(si.on_update)
                    )
                new.append(inst)
            blk.instructions[:] = new


def _host_prep(x, token_positions, Wq, Wk, Wv, Wo):
    B = x.shape[0]
    pos = np.asarray(token_positions, dtype=np.float32)
    inv = (10000.0 ** (-(np.arange(0, HD, 2, dtype=np.float32)) / HD)).astype(
        np.float32
    )
    ang = pos[None, :] * inv[:, None]  # [64, S]
    c, s = np.cos(ang), np.sin(ang)
    cosf = np.empty((HD, S), NPBF16)
    sinsg = np.empty((HD, S), NPBF16)
    cosf[0::2] = c
    cosf[1::2] = c
    sinsg[0::2] = -s
    sinsg[1::2] = s
    pswap = np.zeros((HD, HD), NPBF16)
    idx = np.arange(0, HD, 2)
    pswap[idx, idx + 1] = 1.0
    pswap[idx + 1, idx] = 1.0
    binmask = np.zeros((4 * 128, SB), NPBF16)
    for j in range(4):
        k = np.arange(128)[:, None] + 128 * j
        q = np.arange(SB)[None, :]
        binmask[j * 128 : (j + 1) * 128] = (k <= q).astype(NPBF16)

    scale = np.float32(1.0 / math.sqrt(HD))
    xTs = [np.ascontiguousarray(x[b].T).astype(NPBF16) for b in range(B)]
    in_maps = []
    for c_id in range(8):
        b, g = divmod(c_id, 4)
        cols = slice(DH * g, DH * (g + 1))
        in_maps.append(
            {
                "xT": xTs[b],
                "wqT": np.ascontiguousarray((Wq[cols, :] * scale).T).astype(NPBF16),
                "wkT": np.ascontiguousarray(Wk[cols, :].T).astype(NPBF16),
                "wvT": np.ascontiguousarray(Wv[cols, :].T).astype(NPBF16),
                "woT": np.ascontiguousarray(Wo[:, cols].T).astype(NPBF16),
                "cosf": cosf,
                "sinsg": sinsg,
                "pswap": pswap,
                "binmask": binmask,
            }
        )
    return in_maps


def kernel(x, token_positions, Wq, Wk, Wv, Wo, _trace=False):
    x = np.asarray(x, dtype=np.float32)
    Wq = np.asarray(Wq, dtype=np.float32)
    Wk = np.asarray(Wk, dtype=np.float32)
    Wv = np.asarray(Wv, dtype=np.float32)
    Wo = np.asarray(Wo, dtype=np.float32)
    if "nc" not in _CACHE:
        _CACHE["nc"] = build_bass()
    nc = _CACHE["nc"]
    in_maps = _host_prep(x, token_positions, Wq, Wk, Wv, Wo)
    res = run_bass_kernel_spmd(nc, in_maps, core_ids=list(range(8)), trace=_trace)
    _CACHE["last_result"] = res
    partials = np.stack([r["out"] for r in res.results])  # [8, S, D]
    out = partials.reshape(2, 4, S, D).sum(axis=1)
    return out.astype(np.float32)



# revision 1
# speedup vs baseline: 1.2773x; 1.2773x over previous
"""MHA kernel for TRN2, 8 NeuronCores.

Sharding: core c = b*4 + g handles batch b (of 2) and head-group g (4 of 16
heads, contiguous head-dim columns 512g:512g+512).  Each core computes
  QT/KT = (W[cols,:] @ x_b.T) with RoPE applied   -> [512, 2048] head-dim major
  V     = x_b @ Wv[cols,:].T                      -> [2048, 512]
  causal attention per head in transposed-score layout (no-max softmax;
  scores ~ N(0,1) so exp never overflows)
  partial_out = O_part @ Wo[:, cols].T            -> [2048, 2048]
Host sums the 4 partials per batch.

Matmuls run in bf16 (1 cyc/row on PE); accumulation is fp32 in PSUM.
Elementwise work stays on ACT/DVE only (Pool TT hits the ISA sync-wait
slot limit when an op depends on 3+ engines).
"""

import math

import numpy as np
import ml_dtypes

import concourse.bass as bass
import concourse.mybir as mybir
import concourse.tile as tile
from concourse.bass_utils import run_bass_kernel_spmd

S = 2048
D = 2048
HD = 128  # head dim
NHC = 4  # heads per core
DH = NHC * HD  # 512 head-dim columns per core
NKT = D // 128  # 16 contraction k-tiles
SB = 512  # S block for free dims
NQB = S // SB  # 4 q blocks
F32 = mybir.dt.float32
BF16 = mybir.dt.bfloat16
NPBF16 = ml_dtypes.bfloat16

_CACHE = {}


def build_bass():
    nc = bass.Bass()
    xT = nc.declare_dram_parameter("xT", [D, S], BF16, isOutput=False)
    wqT = nc.declare_dram_parameter("wqT", [D, DH], BF16, isOutput=False)
    wkT = nc.declare_dram_parameter("wkT", [D, DH], BF16, isOutput=False)
    wvT = nc.declare_dram_parameter("wvT", [D, DH], BF16, isOutput=False)
    woT = nc.declare_dram_parameter("woT", [DH, D], BF16, isOutput=False)
    cosf = nc.declare_dram_parameter("cosf", [HD, S], BF16, isOutput=False)
    sinsg = nc.declare_dram_parameter("sinsg", [HD, S], BF16, isOutput=False)
    pswap_d = nc.declare_dram_parameter("pswap", [HD, HD], BF16, isOutput=False)
    binmask_d = nc.declare_dram_parameter(
        "binmask", [4 * 128, SB], BF16, isOutput=False
    )
    out_d = nc.declare_dram_parameter("out", [S, D], F32, isOutput=True)

    with tile.TileContext(nc) as tc:
        with (
            tc.tile_pool(name="psum", bufs=1, space="PSUM") as psum,
            tc.tile_pool(name="main", bufs=1) as mp,
        ):
            # tiny constants first (zero-wait DVE ops at program start)
            ones_col = mp.tile([128, 1], F32, name="ones_col")
            nc.vector.memset(ones_col[:, :], 1.0)
            ones_row = mp.tile([1, 128], F32, name="ones_row")
            nc.vector.memset(ones_row[:, :], 1.0)
            dscr = mp.tile([1, 1], F32, name="dscr")
            _tmpl_dve = nc.vector.memset(dscr[:, :], 0.0)
            _tmpl_act = nc.scalar.copy(dscr[:, :], dscr[:, :])
            _CACHE["tmpl"] = {"DVE": _tmpl_dve.ins, "Activation": _tmpl_act.ins}

            # persistent bf16 tensors: QT/KT per head, V per s-tile, OT per head
            qts = [mp.tile([128, S], BF16, name=f"qt{h}", tag="qt", bufs=NHC)
                   for h in range(NHC)]
            kts = [mp.tile([128, S], BF16, name=f"kt{h}", tag="kt", bufs=NHC)
                   for h in range(NHC)]
            vts = [mp.tile([128, DH], BF16, name=f"v{st}", tag="v", bufs=NKT)
                   for st in range(NKT)]
            ots = [mp.tile([128, S], BF16, name=f"ot{h}", tag="ot", bufs=NHC)
                   for h in range(NHC)]

            # ---------------- phase 1: projections + RoPE ------------------
            with tc.tile_pool(name="ph1", bufs=1) as p1:
                cos_t = p1.tile([HD, S], BF16, name="cos_t")
                sin_t = p1.tile([HD, S], BF16, name="sin_t")
                psw_t = p1.tile([HD, HD], BF16, name="psw_t")
                nc.sync.dma_start(out=cos_t[:, :], in_=cosf[:, :])
                nc.sync.dma_start(out=sin_t[:, :], in_=sinsg[:, :])
                nc.sync.dma_start(out=psw_t[:, :], in_=pswap_d[:, :])
                # DVE touches so later DVE consumers carry own-engine deps
                nc.vector.tensor_copy(cos_t[:, :], cos_t[:, :])
                nc.vector.tensor_copy(sin_t[:, :], sin_t[:, :])

                # xT fully resident: 16 bf16 tiles [128, 2048]
                xts = []
                for kt in range(NKT):
                    xt = p1.tile([128, S], BF16, name=f"xt{kt}", tag="xt", bufs=NKT)
                    nc.sync.dma_start(
                        out=xt[:, :], in_=xT[kt * 128 : (kt + 1) * 128, :]
                    )
                    xts.append(xt)

                # --- V first ---
                wvts = []
                for kt in range(NKT):
                    wv = p1.tile([128, DH], BF16, name=f"wv{kt}", tag="wv", bufs=NKT)
                    nc.sync.dma_start(
                        out=wv[:, :], in_=wvT[kt * 128 : (kt + 1) * 128, :]
                    )
                    wvts.append(wv)
                for st in range(NKT):
                    ps = psum.tile([128, DH], F32, name=f"pv{st}", tag="pA", bufs=3)
                    for kt in range(NKT):
                        nc.tensor.matmul(
                            ps[:, :],
                            xts[kt][:, st * 128 : (st + 1) * 128],
                            wvts[kt][:, :],
                            start=(kt == 0),
                            stop=(kt == NKT - 1),
                        )
                    nc.scalar.copy(vts[st][:, :], ps[:, :])

                # --- Q and K per head: out[hd, S] with RoPE ---
                for h in range(NHC):
                    for proj, wsrc, dsts in (("k", wkT, kts), ("q", wqT, qts)):
                        wt = p1.tile(
                            [128, NKT * 128], BF16, name=f"w_{proj}{h}",
                            tag="wt", bufs=2,
                        )
                        for kt in range(NKT):
                            nc.sync.dma_start(
                                out=wt[:, kt * 128 : (kt + 1) * 128],
                                in_=wsrc[
                                    kt * 128 : (kt + 1) * 128,
                                    h * 128 : (h + 1) * 128,
                                ],
                            )
                        stage = p1.tile(
                            [128, S], BF16, name=f"st_{proj}{h}", tag="stage", bufs=2
                        )
                        for sb in range(NQB):
                            sl = slice(sb * SB, (sb + 1) * SB)
                            ps = psum.tile(
                                [128, SB], F32, name=f"pp{proj}{h}{sb}",
                                tag="pA", bufs=3,
                            )
                            for kt in range(NKT):
                                nc.tensor.matmul(
                                    ps[:, :],
                                    wt[:, kt * 128 : (kt + 1) * 128],
                                    xts[kt][:, sl],
                                    start=(kt == 0),
                                    stop=(kt == NKT - 1),
                                )
                            nc.scalar.copy(stage[:, sl], ps[:, :])
                            # rot = stage*cos + (pswap@stage)*sinsg -> bf16
                            psw = psum.tile(
                                [128, SB], F32, name=f"psw{proj}{h}{sb}",
                                tag="pB", bufs=2,
                            )
                            nc.tensor.matmul(
                                psw[:, :], psw_t[:, :], stage[:, sl],
                                start=True, stop=True,
                            )
                            tmp = p1.tile(
                                [128, SB], F32, name=f"tmp{proj}{h}{sb}",
                                tag="ropetmp", bufs=2,
                            )
                            tsin = p1.tile(
                                [128, SB], F32, name=f"tsin{proj}{h}{sb}",
                                tag="ropetsin", bufs=2,
                            )
                            nc.vector.tensor_tensor(
                                tmp[:, :], stage[:, sl], cos_t[:, sl],
                                mybir.AluOpType.mult,
                            )
                            nc.vector.tensor_tensor(
                                tsin[:, :], psw[:, :], sin_t[:, sl],
                                mybir.AluOpType.mult,
                            )
                            nc.vector.tensor_tensor(
                                dsts[h][:, sl], tsin[:, :], tmp[:, :],
                                mybir.AluOpType.add,
                            )

            # all-engine sync so phase-2 tiles reusing phase-1 addresses
            # don't accumulate per-engine catch-up waits
            tc.strict_bb_all_engine_barrier()

            # ---------------- phase 2: attention per head -------------------
            with tc.tile_pool(name="ph2", bufs=1) as p2:
                masks = []
                for j in range(4):
                    mk = p2.tile([128, SB], BF16, name=f"mask{j}", tag="mask", bufs=4)
                    nc.sync.dma_start(
                        out=mk[:, :], in_=binmask_d[j * 128 : (j + 1) * 128, :]
                    )
                    # DVE touch: later DVE consumers see an own-engine dep
                    nc.vector.tensor_copy(mk[:, :], mk[:, :])
                    masks.append(mk)

                for h in range(NHC):
                    for qb in range(NQB):
                        qsl = slice(qb * SB, (qb + 1) * SB)
                        nkt = 4 * (qb + 1)
                        pot = psum.tile(
                            [128, SB], F32, name=f"pot{h}{qb}", tag="pB", bufs=2
                        )
                        dacc = p2.tile(
                            [128, SB], F32, name=f"dacc{h}{qb}", tag="dacc", bufs=2
                        )
                        for kt in range(nkt):
                            pst = psum.tile(
                                [128, SB], F32, name=f"pst{h}{qb}{kt}",
                                tag="pA", bufs=3,
                            )
                            nc.tensor.matmul(
                                pst[:, :],
                                kts[h][:, kt * 128 : (kt + 1) * 128],
                                qts[h][:, qsl],
                                start=True,
                                stop=True,
                                skip_group_check=True,
                            )
                            es = p2.tile(
                                [128, SB], BF16, name=f"es{h}{qb}{kt}",
                                tag="es", bufs=17,
                            )
                            nc.scalar.activation(
                                es[:, :], pst[:, :], mybir.ActivationFunctionType.Exp
                            )
                            if kt >= 4 * qb:  # diagonal tile -> causal mask
                                nc.vector.tensor_tensor(
                                    es[:, :], es[:, :], masks[kt - 4 * qb][:, :],
                                    mybir.AluOpType.mult,
                                )
                            if kt == 0:
                                nc.vector.tensor_copy(dacc[:, :], es[:, :])
                            else:
                                nc.vector.tensor_tensor(
                                    dacc[:, :], dacc[:, :], es[:, :],
                                    mybir.AluOpType.add,
                                )
                            nc.tensor.matmul(
                                pot[:, :],
                                vts[kt][:, h * 128 : (h + 1) * 128],
                                es[:, :],
                                start=(kt == 0),
                                stop=(kt == nkt - 1),
                                skip_group_check=True,
                            )
                        # denom = colsum(dacc) over partitions -> [1, SB]
                        pden = psum.tile(
                            [1, SB], F32, name=f"pden{h}{qb}", tag="pC", bufs=1
                        )
                        nc.tensor.matmul(
                            pden[:, :], ones_col[:, :], dacc[:, :],
                            start=True, stop=True, skip_group_check=True,
                        )
                        recip = p2.tile(
                            [1, SB], F32, name=f"rc{h}{qb}", tag="recip", bufs=2
                        )
                        nc.vector.reciprocal(recip[:, :], pden[:, :])
                        pbc = psum.tile(
                            [128, SB], F32, name=f"pbc{h}{qb}", tag="pD", bufs=1
                        )
                        nc.tensor.matmul(
                            pbc[:, :], ones_row[:, :], recip[:, :],
                            start=True, stop=True, skip_group_check=True,
                        )
                        nc.scalar.copy(ots[h][:, qsl], pot[:, :])
                        # dummy DVE read of pbc absorbs the PE wait so the
                        # normalize mult only waits on ACT (1-wait TT limit)
                        nc.vector.tensor_copy(dscr[:, :], pbc[0:1, 0:1])
                        nc.vector.tensor_tensor(
                            ots[h][:, qsl], ots[h][:, qsl], pbc[:, :],
                            mybir.AluOpType.mult,
                        )

                # ------------- phase 3: output projection -------------------
                with tc.tile_pool(name="ph3", bufs=1) as p3:
                    wos = []
                    for h in range(NHC):
                        wo = p3.tile([128, D], BF16, name=f"wo{h}", tag="wo", bufs=NHC)
                        nc.sync.dma_start(
                            out=wo[:, :], in_=woT[h * 128 : (h + 1) * 128, :]
                        )
                        wos.append(wo)
                    for st in range(NKT):
                        osb = p3.tile([128, D], F32, name=f"osb{st}", tag="osb", bufs=2)
                        for nb in range(NQB):
                            po = psum.tile(
                                [128, SB], F32, name=f"po{st}{nb}", tag="pA", bufs=3
                            )
                            for h in range(NHC):
                                nc.tensor.matmul(
                                    po[:, :],
                                    ots[h][:, st * 128 : (st + 1) * 128],
                                    wos[h][:, nb * SB : (nb + 1) * SB],
                                    start=(h == 0),
                                    stop=(h == NHC - 1),
                                )
                            nc.scalar.copy(osb[:, nb * SB : (nb + 1) * SB], po[:, :])
                        nc.sync.dma_start(
                            out=out_d[st * 128 : (st + 1) * 128, :], in_=osb[:, :]
                        )
    _legalize_waits(nc)
    return nc


def _legalize_waits(nc):
    """Walrus TT/ACT structs hold only ONE sync wait.  Split excess waits
    onto cloned 1-element carrier ops inserted just before, same queue."""
    import copy

    tmpl = _CACHE["tmpl"]
    n = [0]

    def carrier(eng_name, wait, eng=None):
        n[0] += 1
        if eng_name == "PE":
            c = mybir.InstNoOp(name=f"I-legal-{n[0]}")
            c.engine = eng
        else:
            c = copy.deepcopy(tmpl[eng_name])
            c.name = f"I-legal-{n[0]}"
        c.sync_info = mybir.SyncInfo(on_wait=[wait], on_update=[])
        return c

    for f in nc.m.functions:
        for blk in f.blocks:
            new = []
            for inst in blk.instructions:
                si = getattr(inst, "sync_info", None)
                eng = str(getattr(inst, "engine", ""))
                tname = type(inst).__name__
                if (
                    si is not None
                    and len(si.on_wait) > 1
                    and tname not in ("InstEventSemaphore",)
                ):
                    if "DVE" in eng or "Pool" in eng:
                        key = "DVE"
                    elif "Activation" in eng:
                        key = "Activation"
                    else:
                        key = "PE"
                    waits = list(si.on_wait)
                    for w in waits[:-1]:
                        new.append(carrier(key, w, getattr(inst, "engine", None)))
                    inst.sync_info = mybir.SyncInfo(
                        on_wait=[waits[-1]], on_update=list(si.on_update)
                    )
                new.append(inst)
            blk.instructions[:] = new


def _host_prep(x, token_positions, Wq, Wk, Wv, Wo):
    B = x.shape[0]
    pos = np.asarray(token_positions, dtype=np.float32)
    inv = (10000.0 ** (-(np.arange(0, HD, 2, dtype=np.float32)) / HD)).astype(
        np.float32
    )
    ang = pos[None, :] * inv[:, None]  # [64, S]
    c, s = np.cos(ang), np.sin(ang)
    cosf = np.empty((HD, S), NPBF16)
    sinsg = np.empty((HD, S), NPBF16)
    cosf[0::2] = c
    cosf[1::2] = c
    sinsg[0::2] = -s
    sinsg[1::2] = s
    pswap = np.zeros((HD, HD), NPBF16)
    idx = np.arange(0, HD, 2)
    pswap[idx, idx + 1] = 1.0
    pswap[idx + 1, idx] = 1.0
    binmask = np.zeros((4 * 128, SB), NPBF16)
    for j in range(4):
        k = np.arange(128)[:, None] + 128 * j
        q = np.arange(SB)[None, :]
        binmask[j * 128 : (j + 1) * 128] = (k <= q).astype(NPBF16)

    scale = np.float32(1.0 / math.sqrt(HD))
    xTs = [np.ascontiguousarray(x[b].T).astype(NPBF16) for b in range(B)]
    in_maps = []
    for c_id in range(8):
        b, g = divmod(c_id, 4)
        cols = slice(DH * g, DH * (g + 1))
        in_maps.append(
            {
                "xT": xTs[b],
                "wqT": np.ascontiguousarray((Wq[cols, :] * scale).T).astype(NPBF16),
                "wkT": np.ascontiguousarray(Wk[cols, :].T).astype(NPBF16),
                "wvT": np.ascontiguousarray(Wv[cols, :].T).astype(NPBF16),
                "woT": np.ascontiguousarray(Wo[:, cols].T).astype(NPBF16),
                "cosf": cosf,
                "sinsg": sinsg,
                "pswap": pswap,
                "binmask": binmask,
            }
        )
    return in_maps


def kernel(x, token_positions, Wq, Wk, Wv, Wo, _trace=False):
    x = np.asarray(x, dtype=np.float32)
    Wq = np.asarray(Wq, dtype=np.float32)
    Wk = np.asarray(Wk, dtype=np.float32)
    Wv = np.asarray(Wv, dtype=np.float32)
    Wo = np.asarray(Wo, dtype=np.float32)
    if "nc" not in _CACHE:
        _CACHE["nc"] = build_bass()
    nc = _CACHE["nc"]
    in_maps = _host_prep(x, token_positions, Wq, Wk, Wv, Wo)
    res = run_bass_kernel_spmd(nc, in_maps, core_ids=list(range(8)), trace=_trace)
    _CACHE["last_result"] = res
    partials = np.stack([r["out"] for r in res.results])  # [8, S, D]
    out = partials.reshape(2, 4, S, D).sum(axis=1)
    return out.astype(np.float32)

